# revision 23
# baseline (speedup 1.0000x reference)
"""Trainium2 Bass kernel for nn_CPN_4492535791617 (nms_detection).

Architecture (v2, int16 tables):
  - Detections (N=4096) sharded across 8 cores, 512 each; refinement-derived
    tables replicated per core.
  - Host builds 6 bucket-pair tables T_k[b*8192 + (y*512+x)>>5] = 256B rows,
    int16-quantized (scale folded into the f32 weights), CHANNEL-MAJOR:
    row = [4ch][32px] so the one-hot select multiply has a packed last dim
    (32 px, stride 1) and qualifies for the DVE 2x 16-bit mode. The 4
    channels are [2k, 2k+1, 2(k+1)%6, 2(k+1)%6+1] for floor(6*sampling)=k.
  - Samples host-permuted so equal-bucket samples are contiguous; per-chunk
    gathers run as InstDMAGatherAnt on 4 SWDGE queues.
  - On-chip layout: partition p = n%128, free col = s'*4 + nh (nh = n>>7).
  - Per iteration: round (magic RNE) + clip, pixel/row/unit index math,
    int16 cast, 16-partition index wrap (8 strided SBUF DMAs + 3 doubling
    replicas), gather, one-hot unit-select (int16, exact: one real value +
    zeros) + halving-tree sum over px, weighted 2-term MAC in f32 in
    reference order (dequant via weights).
"""
import numpy as np

import concourse.bacc as bacc
import concourse.mybir as mybir
import concourse.tile as tile
from concourse.bass import AP
from concourse.bass_utils import run_bass_kernel_spmd


def _bc(ap, dims):
    """Build an AP with explicit [step, count] dims after the partition dim."""
    return AP(ap.tensor, ap.offset, [list(ap.ap[0])] + [list(d) for d in dims])

F32 = mybir.dt.float32
F16 = mybir.dt.float16
I16 = mybir.dt.int16
ALU = mybir.AluOpType

H = W = 512
NB = 6                 # buckets
NL = 4                 # refinement iterations
N, S, B = 4096, 128, 4
NCORES = 8
ND = N // NCORES       # 512 detections/core
NH = ND // 128         # 4 n-high values
COLS = S * NH          # 512 free columns: col = s'*4 + nh
ROWS_PER_B = H * W // 32      # 8192 rows per image
TROWS = B * ROWS_PER_B        # 32768 rows per table
UNITS = 32             # px per row
CHUNK_S = 16           # s' per chunk
NCHUNK = S // CHUNK_S  # 8 chunks over all s'
MAGIC = 12582912.0
NQ = 4                 # SWDGE queues

_cache = {}


def _build_nc(bounds):
    """bounds[k] = (start, end) s'-range of bucket k (end-exclusive), on the
    m-sorted sample permutation."""
    nc = bacc.Bacc("TRN2", target_bir_lowering=False, debug=False,
                   num_swdge_queues=NQ, dynamic_dma_scratch_size=32768)

    # const bias APs for the Relu-based round+clip on the ACT engine
    for v in (MAGIC + 511.0, 511.0):
        t = nc.alloc_sbuf_tensor(f"const-f32-{v}", [128, 1], F32)
        nc.gpsimd.memset(t.ap(), v)
        nc.const_aps.aps[(F32, v)] = t.ap()
    nc.all_engine_barrier()

    tbls = [nc.dram_tensor(f"t{k}", [TROWS, 128], F16, kind="ExternalInput")
            for k in range(NB)]
    # iteration-0 prep is host-computed (det_indices are known inputs):
    # rounded/clipped coords, unit index, and pre-wrapped gather rows
    rx0_d = [nc.dram_tensor(f"rx0_{h}", [128, COLS // 2], F32, kind="ExternalInput")
             for h in range(2)]
    ry0_d = [nc.dram_tensor(f"ry0_{h}", [128, COLS // 2], F32, kind="ExternalInput")
             for h in range(2)]
    uu0_d = [nc.dram_tensor(f"uu0_{h}", [128, COLS // 2], F16, kind="ExternalInput")
             for h in range(2)]
    wrp0_d = [nc.dram_tensor(f"wrp0_{h}", [128, COLS * 4], I16, kind="ExternalInput")
              for h in range(2)]
    bB_d = nc.dram_tensor("bB", [128, NH], F32, kind="ExternalInput")
    wv_d = nc.dram_tensor("wv", [128, COLS * 4], F16, kind="ExternalInput")
    iota_d = nc.dram_tensor("iota32", [128, UNITS], F16, kind="ExternalInput")
    outx = nc.dram_tensor("outx", [128, COLS], F32, kind="ExternalOutput")
    outy = nc.dram_tensor("outy", [128, COLS], F32, kind="ExternalOutput")

    # per-chunk gather calls: (bucket k, s'a, s'b) clipped to the chunk
    chunk_calls = []
    for c in range(NCHUNK):
        lo, hi = c * CHUNK_S, (c + 1) * CHUNK_S
        calls = []
        for k in range(NB):
            a, b = bounds[k]
            sa, sb = max(a, lo), min(b, hi)
            if sa < sb:
                calls.append((k, sa, sb))
        chunk_calls.append(calls)

    gather_ord = [0]   # SWDGE DMA ordinal: queue must equal ordinal%NQ so each
                       # round-robined DMASW sem (8 lanes) sees a single queue

    with tile.TileContext(nc) as tc:
        with (
            tc.tile_pool(name="pers", bufs=1) as pers,
            tc.tile_pool(name="work", bufs=2) as work,
            tc.tile_pool(name="cwork", bufs=4) as cwork,
            tc.tile_pool(name="gpool", bufs=4) as gpool,
        ):
            dx = pers.tile([128, COLS], F32, tag="dx")
            dy = pers.tile([128, COLS], F32, tag="dy")
            bB = pers.tile([128, NH], F32, tag="bB")
            wv = pers.tile([128, COLS * 4], F16, tag="wv")
            iota = pers.tile([128, UNITS], F16, tag="iota")
            nc.sync.dma_start(out=bB[:], in_=bB_d.ap())
            nc.sync.dma_start(out=wv[:], in_=wv_d.ap())
            nc.sync.dma_start(out=iota[:], in_=iota_d.ap())

            HS = S // 2            # 64 s' per half
            HCOLS = HS * NH        # 256 cols per half
            CCOLS = CHUNK_S * NH   # 64 cols per chunk

            def prep(hf, first=False):
                """Index math + idx wrap for one half; returns half state."""
                hs0 = hf * HS
                hcol = slice(hs0 * NH, (hs0 + HS) * NH)
                if first:
                    rx = work.tile([128, HCOLS], F32, tag="rx")
                    ry = work.tile([128, HCOLS], F32, tag="ry")
                    uu16 = work.tile([128, HCOLS], F16, tag="uu16")
                    wrp = work.tile([128, HCOLS * 8], I16, tag="wrp")
                    nc.sync.dma_start(out=rx[:], in_=rx0_d[hf].ap())
                    nc.sync.dma_start(out=ry[:], in_=ry0_d[hf].ap())
                    nc.sync.dma_start(out=uu16[:], in_=uu0_d[hf].ap())
                    nc.sync.dma_start(out=wrp[:], in_=wrp0_d[hf].ap())
                    return dict(hf=hf, hs0=hs0, rx=rx, ry=ry, uu16=uu16, wrp=wrp)
                rx = work.tile([128, HCOLS], F32, tag="rx")
                ry = work.tile([128, HCOLS], F32, tag="ry")
                px = work.tile([128, HCOLS], F32, tag="px")
                fl = work.tile([128, HCOLS], F32, tag="fl")
                rowf = work.tile([128, HCOLS], F32, tag="rowf")
                uu = work.tile([128, HCOLS], F32, tag="uu")
                uu16 = work.tile([128, HCOLS], F16, tag="uu16")
                idx = work.tile([128, HCOLS], I16, tag="idx")
                wrp = work.tile([128, HCOLS * 8], I16, tag="wrp")  # wrapped idx

                # round-half-even + clip entirely on ACT:
                #   a = x + MAGIC          (RNE to integer, biased by MAGIC)
                #   c = relu(-a + MAGIC + 511) = relu(511 - round(x))
                #   r = relu(-c + 511)     = clip(round(x), 0, 511)
                ACTF = mybir.ActivationFunctionType.Copy
                RELU = mybir.ActivationFunctionType.Relu
                nc.scalar.activation(out=rx[:], in_=dx[:, hcol], func=ACTF, bias=MAGIC)
                nc.scalar.activation(out=rx[:], in_=rx[:], func=RELU,
                                     scale=-1.0, bias=MAGIC + 511.0)
                nc.scalar.activation(out=rx[:], in_=rx[:], func=RELU,
                                     scale=-1.0, bias=511.0)
                nc.scalar.activation(out=ry[:], in_=dy[:, hcol], func=ACTF, bias=MAGIC)
                nc.scalar.activation(out=ry[:], in_=ry[:], func=RELU,
                                     scale=-1.0, bias=MAGIC + 511.0)
                nc.scalar.activation(out=ry[:], in_=ry[:], func=RELU,
                                     scale=-1.0, bias=511.0)
                # px = y*512 + x ; fl = floor(px/32) ; row = fl + b*8192
                # u = px - 32*fl          (all exact in f32)
                nc.vector.scalar_tensor_tensor(out=px[:], in0=ry[:], scalar=512.0,
                                               in1=rx[:], op0=ALU.mult, op1=ALU.add)
                nc.scalar.activation(out=fl[:], in_=px[:], func=ACTF,
                                     scale=0.03125, bias=-0.484375)
                nc.scalar.activation(out=fl[:], in_=fl[:], func=ACTF, bias=MAGIC)
                nc.scalar.activation(out=fl[:], in_=fl[:], func=ACTF, bias=-MAGIC)
                bBb = _bc(bB[:], [[0, HS], [1, NH]])
                nc.vector.tensor_tensor(out=rowf[:].rearrange("p (s nh) -> p s nh", nh=NH),
                                        in0=fl[:].rearrange("p (s nh) -> p s nh", nh=NH),
                                        in1=bBb, op=ALU.add)
                nc.vector.scalar_tensor_tensor(out=uu[:], in0=fl[:], scalar=-32.0,
                                               in1=px[:], op0=ALU.mult, op1=ALU.add)
                nc.scalar.activation(out=idx[:], in_=rowf[:], func=ACTF)
                nc.scalar.activation(out=uu16[:], in_=uu[:], func=ACTF)

                # wrap idx into [16, HCOLS*8] layout expected by dma_gather:
                # wrapped[p16, s'l*32 + nh*8 + sh] = idx[sh*16+p16, s'l*4+nh]
                wv8 = wrp[:].rearrange("p (s nh sh) -> p s nh sh", nh=NH, sh=8)
                for sh in range(8):
                    nc.sync.dma_start(
                        out=wv8[0:16, :, :, sh],
                        in_=idx[sh * 16:(sh + 1) * 16, :]
                            .rearrange("p (s nh) -> p s nh", nh=NH))
                for r in (1, 2, 4):   # doubling replication 16->32->64->128
                    nc.sync.dma_start(out=wrp[r * 16:2 * r * 16, :],
                                      in_=wrp[0:r * 16, :])
                return dict(hf=hf, hs0=hs0, rx=rx, ry=ry, uu16=uu16, wrp=wrp)

            chunk_ord = [0]    # global chunk ordinal for engine load-balance
            pending_tail = [None]

            def run_chunks(st):
                hf, hs0 = st["hf"], st["hs0"]
                rx, ry, uu16, wrp = st["rx"], st["ry"], st["uu16"], st["wrp"]
                first_c = hf * NCHUNK // 2
                last_c = (hf + 1) * NCHUNK // 2 - 1
                for c in range(first_c, last_c + 1):
                    # Pool cannot run is_equal (HW engine check); the one-hot
                    # builds on DVE, and the add-only tree runs on Pool for a
                    # fraction of chunks to balance engine load
                    tree_on_pool = chunk_ord[0] % 8 in (1, 4, 6)
                    chunk_ord[0] += 1
                    sel_eng = nc.vector
                    oh_eng = nc.vector
                    tree_eng = nc.gpsimd if tree_on_pool else nc.vector
                    cs = c * CHUNK_S
                    col0 = cs * NH                      # first col of chunk
                    g = gpool.tile([128, CCOLS * 128], F16, tag="g")
                    ohv = cwork.tile([128, CCOLS * UNITS], F16, tag="ohv")
                    sel = cwork.tile([128, CCOLS * 4], F32, tag="sel")
                    resp = cwork.tile([128, CCOLS * 2], F32, tag="resp")

                    for (k, sa, sb) in chunk_calls[c]:
                        nidx = (sb - sa) * NH * 128
                        nc.gpsimd.dma_gather(
                            out_ap=g[:, (sa - cs) * NH * 128:(sb - cs) * NH * 128]
                                  .rearrange("p (n e) -> p n e", e=128),
                            in_ap=tbls[k].ap(),
                            idxs_ap=wrp[:, (sa - hs0) * UNITS:(sb - hs0) * UNITS],
                            num_idxs=nidx, num_idxs_reg=nidx, elem_size=128,
                            single_packet=False,
                            queue_num=gather_ord[0] % NQ)
                        gather_ord[0] += 1

                    # previous chunk's tail goes on Pool here, after this
                    # chunk's desc-gen (tails stay within their half so the
                    # next prep's dx/dy reads are emitted after all writers)
                    if pending_tail[0] is not None:
                        pending_tail[0]()
                        pending_tail[0] = None
                    # one-hot of u over the 32 px (int16; row is [4ch][32px]);
                    # built on the Pool engine to keep DVE on the select path
                    lcol0 = col0 - hs0 * NH             # col within the half
                    ohview = ohv[:].rearrange("p (col u) -> p col u", u=UNITS)
                    ucols = uu16[:, lcol0:lcol0 + CCOLS]
                    oh_eng.tensor_tensor(
                        out=ohview,
                        in0=ucols.to_broadcast([128, CCOLS, UNITS]),
                        in1=_bc(iota[:], [[0, CCOLS], [1, UNITS]]),
                        op=ALU.is_equal)
                    # select the point's px: multiply by one-hot (packed last
                    # dim -> 2x 16-bit mode), halving-tree sum over px (exact)
                    g4 = g[:].rearrange("p (col j u) -> p col j u", j=4, u=UNITS)
                    sel_eng.tensor_tensor(
                        out=g4, in0=g4,
                        in1=_bc(ohv[:], [[UNITS, CCOLS], [0, 4], [1, UNITS]]),
                        op=ALU.mult)
                    h = UNITS // 2
                    while h >= 1:
                        tree_eng.tensor_tensor(
                            out=g4[:, :, :, 0:h], in0=g4[:, :, :, 0:h],
                            in1=g4[:, :, :, h:2 * h], op=ALU.add)
                        h //= 2

                    def tail(g4=g4, sel=sel, resp=resp, rx=rx, ry=ry,
                             col0=col0, lcol0=lcol0, eng=nc.gpsimd):
                        # sel = unit * [w0,w0,w1,w1]/qscale; resp = pair sums;
                        # det = rounded + responses  (small f32 ops on Pool)
                        eng.tensor_tensor(
                            out=sel[:].rearrange("p (col j) -> p col j", j=4),
                            in0=g4[:, :, :, 0],
                            in1=wv[:, col0 * 4:(col0 + CCOLS) * 4]
                                .rearrange("p (col j) -> p col j", j=4),
                            op=ALU.mult)
                        s4 = sel[:].rearrange("p (col k j) -> p col k j", k=2, j=2)
                        eng.tensor_tensor(
                            out=resp[:].rearrange("p (col j) -> p col j", j=2),
                            in0=s4[:, :, 0, :], in1=s4[:, :, 1, :], op=ALU.add)
                        r2 = resp[:].rearrange("p (col j) -> p col j", j=2)
                        ccols = slice(col0, col0 + CCOLS)
                        lcols = slice(lcol0, lcol0 + CCOLS)
                        eng.tensor_tensor(out=dx[:, ccols], in0=rx[:, lcols],
                                           in1=r2[:, :, 0], op=ALU.add)
                        eng.tensor_tensor(out=dy[:, ccols], in0=ry[:, lcols],
                                          in1=r2[:, :, 1], op=ALU.add)
                    pending_tail[0] = tail
                pending_tail[0]()
                pending_tail[0] = None

            # software-pipelined schedule: prep(i+2) is emitted right after
            # chunks(i) (its dx/dy-half writer) so its index math + idx-wrap
            # DMAs + first gathers overlap chunks(i+1)'s selects.
            halves = [hf for _ in range(NL) for hf in (0, 1)]
            states = [prep(halves[0], first=True), prep(halves[1], first=True)]
            for i in range(len(halves)):
                run_chunks(states[i])
                if i + 2 < len(halves):
                    states.append(prep(halves[i + 2]))

            nc.sync.dma_start(out=outx.ap(), in_=dx[:])
            nc.sync.dma_start(out=outy.ap(), in_=dy[:])

    nc.compile()
    return nc


def prep_inputs(det_indices, refinement, sampling, b):
    det = np.asarray(det_indices, np.float32)
    ref = np.asarray(refinement, np.float32)
    samp = np.asarray(sampling, np.float32)
    bb = np.asarray(b).astype(np.int64)

    base = samp * np.float32(NB)
    m = np.floor(base).astype(np.int32)                    # [S]
    wts = {}
    for off in (0, 1):
        d = np.abs((m + off).astype(np.float32) - base)
        wts[off] = np.where(d > 1.0, np.float32(0),
                            np.float32(1) - d).astype(np.float32)

    sperm = np.argsort(m, kind="stable")                   # s' -> s
    msort = m[sperm]
    bounds = []
    for k in range(NB):
        idxs = np.nonzero(msort == k)[0]
        bounds.append((int(idxs[0]), int(idxs[-1]) + 1) if len(idxs)
                      else (0, 0))

    # tables: channel-major rows [4ch][32px], float16:
    # T_k[b*8192 + px>>5, j*32 + (px&31)] = f16(ref_t[b, y, x, sel_k[j]])
    ref_t = np.transpose(ref, (0, 2, 3, 1))                # [B,H,W,12]
    ref_q = ref_t.astype(np.float16)
    tables = {}
    for k in range(NB):
        k1 = (k + 1) % NB
        sel = [2 * k, 2 * k + 1, 2 * k1, 2 * k1 + 1]
        # [B,H,16blk,32u,4j] -> [B,H,blk,4j,32u] -> [TROWS, 128]
        tk = ref_q[:, :, :, sel].reshape(B, H, W // UNITS, UNITS, 4)
        tk = np.ascontiguousarray(tk.transpose(0, 1, 2, 4, 3))
        tables[f"t{k}"] = tk.reshape(TROWS, 128)

    # weights along m-sorted cols: wv[p, (s'*4+nh)*4 + c] = [w0,w0,w1,w1]
    w0s, w1s = wts[0][sperm], wts[1][sperm]                # [S] in s' order
    wvrow = np.stack([w0s, w0s, w1s, w1s], -1)             # [S,4]
    wvrow = np.repeat(wvrow[:, None, :], NH, axis=1).reshape(-1)   # [S*NH*4]
    wvfull = np.ascontiguousarray(np.broadcast_to(wvrow, (128, COLS * 4))
                                  .astype(np.float16))

    iota32 = np.ascontiguousarray(
        np.broadcast_to(np.arange(UNITS, dtype=np.float16), (128, UNITS)))

    in_maps = []
    HCOLS = COLS // 2
    for core in range(NCORES):
        n0 = core * ND
        dsl = det[n0:n0 + ND]                              # [ND,S,2]
        # [p, s'*4+nh] = det[n0 + nh*128 + p, sperm[s'], j]
        dn = dsl.reshape(NH, 128, S, 2)[:, :, sperm, :]    # [nh,p,s',2]
        dxh = np.ascontiguousarray(dn[..., 0].transpose(1, 2, 0).reshape(128, COLS))
        dyh = np.ascontiguousarray(dn[..., 1].transpose(1, 2, 0).reshape(128, COLS))
        bco = bb[n0:n0 + ND].reshape(NH, 128)              # [nh,p]
        bBh = np.ascontiguousarray(
            (bco.T * ROWS_PER_B).astype(np.float32))       # [p,nh]

        # host-computed iteration-0 prep (round RNE + clip + index math +
        # the 16-partition wrapped/replicated gather-row layout)
        rx0 = np.clip(np.rint(dxh.astype(np.float32)), 0, W - 1).astype(np.float32)
        ry0 = np.clip(np.rint(dyh.astype(np.float32)), 0, H - 1).astype(np.float32)
        px0 = ry0 * np.float32(W) + rx0
        fl0 = np.floor(px0 / 32).astype(np.float32)
        row0 = fl0 + bBh[:, np.tile(np.arange(NH), S)]     # [p, col]
        uu0 = (px0 - 32 * fl0).astype(np.float16)
        idx0 = row0.astype(np.int16)
        cmap = {}
        for hf in range(2):
            sl = slice(hf * HCOLS, (hf + 1) * HCOLS)
            idxh = idx0[:, sl]                             # [128, HCOLS]
            wrp = np.empty((16, HCOLS * 8), np.int16)
            for sh in range(8):
                wrp[:, sh::8] = idxh[sh * 16:(sh + 1) * 16, :]
            cmap[f"rx0_{hf}"] = np.ascontiguousarray(rx0[:, sl])
            cmap[f"ry0_{hf}"] = np.ascontiguousarray(ry0[:, sl])
            cmap[f"uu0_{hf}"] = np.ascontiguousarray(uu0[:, sl])
            cmap[f"wrp0_{hf}"] = np.ascontiguousarray(np.tile(wrp, (8, 1)))
        in_maps.append({**tables, **cmap,
                        "bB": bBh, "wv": wvfull, "iota32": iota32})
    return in_maps, tuple(bounds), sperm


def assemble_output(results, sperm):
    inv = np.empty_like(sperm)
    inv[sperm] = np.arange(S)
    out = np.empty((N, S, 2), np.float32)
    for core in range(NCORES):
        n0 = core * ND
        ox = results[core]["outx"].reshape(128, S, NH)     # [p,s',nh]
        oy = results[core]["outy"].reshape(128, S, NH)
        # out[n0 + nh*128 + p, s, j]
        out[n0:n0 + ND, :, 0] = ox.transpose(2, 0, 1).reshape(ND, S)[:, inv]
        out[n0:n0 + ND, :, 1] = oy.transpose(2, 0, 1).reshape(ND, S)[:, inv]
    return out


def get_nc(bounds):
    if bounds not in _cache:
        _cache[bounds] = _build_nc(bounds)
    return _cache[bounds]


def kernel(det_indices, refinement, sampling, b,
           num_loops=4, num_buckets=6, height=512, width=512, **_kw):
    assert int(num_loops) == NL and int(num_buckets) == NB
    assert int(height) == H and int(width) == W
    in_maps, bounds, sperm = prep_inputs(det_indices, refinement, sampling, b)
    nc = get_nc(bounds)
    res = run_bass_kernel_spmd(nc, in_maps, core_ids=list(range(NCORES)))
    return assemble_output(res.results, sperm)


# revision 34
# speedup vs baseline: 1.0141x; 1.0141x over previous
"""Trainium2 Bass kernel for nn_CPN_4492535791617 (nms_detection).

Architecture (v2, int16 tables):
  - Detections (N=4096) sharded across 8 cores, 512 each; refinement-derived
    tables replicated per core.
  - Host builds 6 bucket-pair tables T_k[b*8192 + (y*512+x)>>5] = 256B rows,
    int16-quantized (scale folded into the f32 weights), CHANNEL-MAJOR:
    row = [4ch][32px] so the one-hot select multiply has a packed last dim
    (32 px, stride 1) and qualifies for the DVE 2x 16-bit mode. The 4
    channels are [2k, 2k+1, 2(k+1)%6, 2(k+1)%6+1] for floor(6*sampling)=k.
  - Samples host-permuted so equal-bucket samples are contiguous; per-chunk
    gathers run as InstDMAGatherAnt on 4 SWDGE queues.
  - On-chip layout: partition p = n%128, free col = s'*4 + nh (nh = n>>7).
  - Per iteration: round (magic RNE) + clip, pixel/row/unit index math,
    int16 cast, 16-partition index wrap (8 strided SBUF DMAs + 3 doubling
    replicas), gather, one-hot unit-select (int16, exact: one real value +
    zeros) + halving-tree sum over px, weighted 2-term MAC in f32 in
    reference order (dequant via weights).
"""
import numpy as np

import concourse.bacc as bacc
import concourse.mybir as mybir
import concourse.tile as tile
from concourse.bass import AP
from concourse.bass_utils import run_bass_kernel_spmd


def _bc(ap, dims):
    """Build an AP with explicit [step, count] dims after the partition dim."""
    return AP(ap.tensor, ap.offset, [list(ap.ap[0])] + [list(d) for d in dims])

F32 = mybir.dt.float32
F16 = mybir.dt.float16
I16 = mybir.dt.int16
ALU = mybir.AluOpType

H = W = 512
NB = 6                 # buckets
NL = 4                 # refinement iterations
N, S, B = 4096, 128, 4
NCORES = 8
ND = N // NCORES       # 512 detections/core
NH = ND // 128         # 4 n-high values
COLS = S * NH          # 512 free columns: col = s'*4 + nh
ROWS_PER_B = H * W // 32      # 8192 rows per image
TROWS = B * ROWS_PER_B        # 32768 rows per table
UNITS = 32             # px per row
CHUNK_S = 16           # s' per chunk
NCHUNK = S // CHUNK_S  # 8 chunks over all s'
MAGIC = 12582912.0
NQ = 4                 # SWDGE queues

_cache = {}


def _build_nc(bounds):
    """bounds[k] = (start, end) s'-range of bucket k (end-exclusive), on the
    m-sorted sample permutation."""
    nc = bacc.Bacc("TRN2", target_bir_lowering=False, debug=False,
                   num_swdge_queues=NQ, dynamic_dma_scratch_size=32768)

    # const bias APs for the Relu-based round+clip on the ACT engine
    for v in (MAGIC + 511.0, 511.0):
        t = nc.alloc_sbuf_tensor(f"const-f32-{v}", [128, 1], F32)
        nc.gpsimd.memset(t.ap(), v)
        nc.const_aps.aps[(F32, v)] = t.ap()
    nc.all_engine_barrier()

    tbls = [nc.dram_tensor(f"t{k}", [TROWS, 128], F16, kind="ExternalInput")
            for k in range(NB)]
    # iteration-0 prep is host-computed (det_indices are known inputs):
    # rounded/clipped coords, unit index, and pre-wrapped gather rows
    rx0_d = [nc.dram_tensor(f"rx0_{h}", [128, COLS // 2], F32, kind="ExternalInput")
             for h in range(2)]
    ry0_d = [nc.dram_tensor(f"ry0_{h}", [128, COLS // 2], F32, kind="ExternalInput")
             for h in range(2)]
    uu0_d = [nc.dram_tensor(f"uu0_{h}", [128, COLS // 2], F16, kind="ExternalInput")
             for h in range(2)]
    wrp0_d = [nc.dram_tensor(f"wrp0_{h}", [128, COLS * 4], I16, kind="ExternalInput")
              for h in range(2)]
    bB_d = nc.dram_tensor("bB", [128, NH], F32, kind="ExternalInput")
    wv_d = nc.dram_tensor("wv", [128, COLS * 4], F16, kind="ExternalInput")
    iota_d = nc.dram_tensor("iota32", [128, UNITS], F16, kind="ExternalInput")
    outx = nc.dram_tensor("outx", [128, COLS], F32, kind="ExternalOutput")
    outy = nc.dram_tensor("outy", [128, COLS], F32, kind="ExternalOutput")

    # per-chunk gather calls: (bucket k, s'a, s'b) clipped to the chunk
    chunk_calls = []
    for c in range(NCHUNK):
        lo, hi = c * CHUNK_S, (c + 1) * CHUNK_S
        calls = []
        for k in range(NB):
            a, b = bounds[k]
            sa, sb = max(a, lo), min(b, hi)
            if sa < sb:
                calls.append((k, sa, sb))
        chunk_calls.append(calls)

    gather_ord = [0]   # SWDGE DMA ordinal: queue must equal ordinal%NQ so each
                       # round-robined DMASW sem (8 lanes) sees a single queue

    with tile.TileContext(nc) as tc:
        with (
            tc.tile_pool(name="pers", bufs=1) as pers,
            tc.tile_pool(name="work", bufs=3) as work,
            tc.tile_pool(name="cwork", bufs=6) as cwork,
            tc.tile_pool(name="gpool", bufs=4) as gpool,
        ):
            dx = pers.tile([128, COLS], F32, tag="dx")
            dy = pers.tile([128, COLS], F32, tag="dy")
            bB = pers.tile([128, NH], F32, tag="bB")
            wv = pers.tile([128, COLS * 4], F16, tag="wv")
            iota = pers.tile([128, UNITS], F16, tag="iota")
            nc.sync.dma_start(out=bB[:], in_=bB_d.ap())
            nc.sync.dma_start(out=wv[:], in_=wv_d.ap())
            nc.sync.dma_start(out=iota[:], in_=iota_d.ap())

            HS = S // 2            # 64 s' per half
            HCOLS = HS * NH        # 256 cols per half
            CCOLS = CHUNK_S * NH   # 64 cols per chunk

            def make_ureps(uu16):
                """Per-chunk broadcast copies of u on ACT so the DVE
                is_equal sees packed APs (2x 16-bit mode)."""
                ureps = []
                for q in range(NCHUNK // 2):
                    ur = cwork.tile([128, CCOLS * UNITS], F16, tag="urep")
                    nc.scalar.activation(
                        out=ur[:].rearrange("p (col u) -> p col u", u=UNITS),
                        in_=uu16[:, q * CCOLS:(q + 1) * CCOLS]
                            .to_broadcast([128, CCOLS, UNITS]),
                        func=mybir.ActivationFunctionType.Copy)
                    ureps.append(ur)
                return ureps

            def prep(hf, first=False):
                """Index math + idx wrap for one half; returns half state."""
                hs0 = hf * HS
                hcol = slice(hs0 * NH, (hs0 + HS) * NH)
                if first:
                    rx = work.tile([128, HCOLS], F32, tag="rx")
                    ry = work.tile([128, HCOLS], F32, tag="ry")
                    uu16 = work.tile([128, HCOLS], F16, tag="uu16")
                    wrp = work.tile([128, HCOLS * 8], I16, tag="wrp")
                    nc.sync.dma_start(out=rx[:], in_=rx0_d[hf].ap())
                    nc.sync.dma_start(out=ry[:], in_=ry0_d[hf].ap())
                    nc.sync.dma_start(out=uu16[:], in_=uu0_d[hf].ap())
                    nc.sync.dma_start(out=wrp[:], in_=wrp0_d[hf].ap())
                    return dict(hf=hf, hs0=hs0, rx=rx, ry=ry, uu16=uu16,
                                wrp=wrp, urep=make_ureps(uu16))
                rx = work.tile([128, HCOLS], F32, tag="rx")
                ry = work.tile([128, HCOLS], F32, tag="ry")
                px = work.tile([128, HCOLS], F32, tag="px")
                fl = work.tile([128, HCOLS], F32, tag="fl")
                rowf = work.tile([128, HCOLS], F32, tag="rowf")
                uu = work.tile([128, HCOLS], F32, tag="uu")
                uu16 = work.tile([128, HCOLS], F16, tag="uu16")
                idx = work.tile([128, HCOLS], I16, tag="idx")
                wrp = work.tile([128, HCOLS * 8], I16, tag="wrp")  # wrapped idx

                # round-half-even + clip entirely on ACT:
                #   a = x + MAGIC          (RNE to integer, biased by MAGIC)
                #   c = relu(-a + MAGIC + 511) = relu(511 - round(x))
                #   r = relu(-c + 511)     = clip(round(x), 0, 511)
                ACTF = mybir.ActivationFunctionType.Copy
                RELU = mybir.ActivationFunctionType.Relu
                nc.scalar.activation(out=rx[:], in_=dx[:, hcol], func=ACTF, bias=MAGIC)
                nc.scalar.activation(out=rx[:], in_=rx[:], func=RELU,
                                     scale=-1.0, bias=MAGIC + 511.0)
                nc.scalar.activation(out=rx[:], in_=rx[:], func=RELU,
                                     scale=-1.0, bias=511.0)
                nc.scalar.activation(out=ry[:], in_=dy[:, hcol], func=ACTF, bias=MAGIC)
                nc.scalar.activation(out=ry[:], in_=ry[:], func=RELU,
                                     scale=-1.0, bias=MAGIC + 511.0)
                nc.scalar.activation(out=ry[:], in_=ry[:], func=RELU,
                                     scale=-1.0, bias=511.0)
                # px = y*512 + x ; fl = floor(px/32) ; row = fl + b*8192
                # u = px - 32*fl          (all exact in f32)
                nc.vector.scalar_tensor_tensor(out=px[:], in0=ry[:], scalar=512.0,
                                               in1=rx[:], op0=ALU.mult, op1=ALU.add)
                nc.scalar.activation(out=fl[:], in_=px[:], func=ACTF,
                                     scale=0.03125, bias=-0.484375)
                nc.scalar.activation(out=fl[:], in_=fl[:], func=ACTF, bias=MAGIC)
                nc.scalar.activation(out=fl[:], in_=fl[:], func=ACTF, bias=-MAGIC)
                bBb = _bc(bB[:], [[0, HS], [1, NH]])
                nc.vector.tensor_tensor(out=rowf[:].rearrange("p (s nh) -> p s nh", nh=NH),
                                        in0=fl[:].rearrange("p (s nh) -> p s nh", nh=NH),
                                        in1=bBb, op=ALU.add)
                nc.vector.scalar_tensor_tensor(out=uu[:], in0=fl[:], scalar=-32.0,
                                               in1=px[:], op0=ALU.mult, op1=ALU.add)
                nc.scalar.activation(out=idx[:], in_=rowf[:], func=ACTF)
                nc.scalar.activation(out=uu16[:], in_=uu[:], func=ACTF)

                # wrap idx into [16, HCOLS*8] layout expected by dma_gather:
                # wrapped[p16, s'l*32 + nh*8 + sh] = idx[sh*16+p16, s'l*4+nh]
                wv8 = wrp[:].rearrange("p (s nh sh) -> p s nh sh", nh=NH, sh=8)
                for sh in range(8):
                    nc.sync.dma_start(
                        out=wv8[0:16, :, :, sh],
                        in_=idx[sh * 16:(sh + 1) * 16, :]
                            .rearrange("p (s nh) -> p s nh", nh=NH))
                for r in (1, 2, 4):   # doubling replication 16->32->64->128
                    nc.sync.dma_start(out=wrp[r * 16:2 * r * 16, :],
                                      in_=wrp[0:r * 16, :])
                return dict(hf=hf, hs0=hs0, rx=rx, ry=ry, uu16=uu16,
                            wrp=wrp, urep=make_ureps(uu16))

            chunk_ord = [0]    # global chunk ordinal for engine load-balance
            pending_tail = [None]

            def run_chunks(st):
                hf, hs0 = st["hf"], st["hs0"]
                rx, ry, uu16, wrp = st["rx"], st["ry"], st["uu16"], st["wrp"]
                first_c = hf * NCHUNK // 2
                last_c = (hf + 1) * NCHUNK // 2 - 1
                for c in range(first_c, last_c + 1):
                    # Pool cannot run is_equal (HW engine check); the one-hot
                    # builds on DVE, and the add-only tree runs on Pool for a
                    # fraction of chunks to balance engine load
                    tree_on_pool = chunk_ord[0] % 8 in (1, 5)
                    chunk_ord[0] += 1
                    sel_eng = nc.vector
                    oh_eng = nc.vector
                    tree_eng = nc.gpsimd if tree_on_pool else nc.vector
                    cs = c * CHUNK_S
                    col0 = cs * NH                      # first col of chunk
                    gt = gpool.tile([128, CCOLS * 128], F16, tag="g")
                    for (k, sa, sb) in chunk_calls[c]:
                        nidx = (sb - sa) * NH * 128
                        nc.gpsimd.dma_gather(
                            out_ap=gt[:, (sa - cs) * NH * 128:
                                      (sb - cs) * NH * 128]
                                  .rearrange("p (n e) -> p n e", e=128),
                            in_ap=tbls[k].ap(),
                            idxs_ap=wrp[:, (sa - hs0) * UNITS:
                                        (sb - hs0) * UNITS],
                            num_idxs=nidx, num_idxs_reg=nidx, elem_size=128,
                            single_packet=False,
                            queue_num=gather_ord[0] % NQ)
                        gather_ord[0] += 1
                    g = gt[:]
                    ohv = cwork.tile([128, CCOLS * UNITS], F16, tag="ohv")
                    sel = cwork.tile([128, CCOLS * 4], F32, tag="sel")
                    resp = cwork.tile([128, CCOLS * 2], F32, tag="resp")

                    # previous chunk's tail goes on Pool here, after this
                    # chunk's desc-gen (tails stay within their half so the
                    # next prep's dx/dy reads are emitted after all writers)
                    if pending_tail[0] is not None:
                        pending_tail[0]()
                        pending_tail[0] = None
                    # one-hot of u over the 32 px (int16; row is [4ch][32px]);
                    # built on the Pool engine to keep DVE on the select path
                    lcol0 = col0 - hs0 * NH             # col within the half
                    ohview = ohv[:].rearrange("p (col u) -> p col u", u=UNITS)
                    ucols = uu16[:, lcol0:lcol0 + CCOLS]
                    oh_eng.tensor_tensor(
                        out=ohview,
                        in0=st["urep"][(c - first_c)][:]
                            .rearrange("p (col u) -> p col u", u=UNITS),
                        in1=_bc(iota[:], [[0, CCOLS], [1, UNITS]]),
                        op=ALU.is_equal)
                    # select the point's px: multiply by one-hot (packed last
                    # dim -> 2x 16-bit mode), halving-tree sum over px (exact)
                    g4 = g.rearrange("p (col j u) -> p col j u", j=4, u=UNITS)
                    sel_eng.tensor_tensor(
                        out=g4, in0=g4,
                        in1=_bc(ohv[:], [[UNITS, CCOLS], [0, 4], [1, UNITS]]),
                        op=ALU.mult)
                    h = UNITS // 2
                    while h >= 1:
                        tree_eng.tensor_tensor(
                            out=g4[:, :, :, 0:h], in0=g4[:, :, :, 0:h],
                            in1=g4[:, :, :, h:2 * h], op=ALU.add)
                        h //= 2

                    def tail(g4=g4, sel=sel, resp=resp, rx=rx, ry=ry,
                             col0=col0, lcol0=lcol0, eng=nc.gpsimd):
                        # sel = unit * [w0,w0,w1,w1]/qscale; resp = pair sums;
                        # det = rounded + responses  (small f32 ops on Pool)
                        eng.tensor_tensor(
                            out=sel[:].rearrange("p (col j) -> p col j", j=4),
                            in0=g4[:, :, :, 0],
                            in1=wv[:, col0 * 4:(col0 + CCOLS) * 4]
                                .rearrange("p (col j) -> p col j", j=4),
                            op=ALU.mult)
                        s4 = sel[:].rearrange("p (col k j) -> p col k j", k=2, j=2)
                        eng.tensor_tensor(
                            out=resp[:].rearrange("p (col j) -> p col j", j=2),
                            in0=s4[:, :, 0, :], in1=s4[:, :, 1, :], op=ALU.add)
                        r2 = resp[:].rearrange("p (col j) -> p col j", j=2)
                        ccols = slice(col0, col0 + CCOLS)
                        lcols = slice(lcol0, lcol0 + CCOLS)
                        eng.tensor_tensor(out=dx[:, ccols], in0=rx[:, lcols],
                                           in1=r2[:, :, 0], op=ALU.add)
                        eng.tensor_tensor(out=dy[:, ccols], in0=ry[:, lcols],
                                          in1=r2[:, :, 1], op=ALU.add)
                    pending_tail[0] = tail
                pending_tail[0]()
                pending_tail[0] = None

            # software-pipelined schedule: prep(i+2) is emitted right after
            # chunks(i) (its dx/dy-half writer) so its index math + idx-wrap
            # DMAs + first gathers overlap chunks(i+1)'s selects.
            halves = [hf for _ in range(NL) for hf in (0, 1)]
            states = [prep(halves[0], first=True), prep(halves[1], first=True)]
            for i in range(len(halves)):
                run_chunks(states[i])
                if i + 2 < len(halves):
                    states.append(prep(halves[i + 2]))

            nc.sync.dma_start(out=outx.ap(), in_=dx[:])
            nc.sync.dma_start(out=outy.ap(), in_=dy[:])

    nc.compile()
    return nc


def prep_inputs(det_indices, refinement, sampling, b):
    det = np.asarray(det_indices, np.float32)
    ref = np.asarray(refinement, np.float32)
    samp = np.asarray(sampling, np.float32)
    bb = np.asarray(b).astype(np.int64)

    base = samp * np.float32(NB)
    m = np.floor(base).astype(np.int32)                    # [S]
    wts = {}
    for off in (0, 1):
        d = np.abs((m + off).astype(np.float32) - base)
        wts[off] = np.where(d > 1.0, np.float32(0),
                            np.float32(1) - d).astype(np.float32)

    sperm = np.argsort(m, kind="stable")                   # s' -> s
    msort = m[sperm]
    bounds = []
    for k in range(NB):
        idxs = np.nonzero(msort == k)[0]
        bounds.append((int(idxs[0]), int(idxs[-1]) + 1) if len(idxs)
                      else (0, 0))

    # tables: channel-major rows [4ch][32px], float16:
    # T_k[b*8192 + px>>5, j*32 + (px&31)] = f16(ref_t[b, y, x, sel_k[j]])
    ref_t = np.transpose(ref, (0, 2, 3, 1))                # [B,H,W,12]
    ref_q = ref_t.astype(np.float16)
    tables = {}
    for k in range(NB):
        k1 = (k + 1) % NB
        sel = [2 * k, 2 * k + 1, 2 * k1, 2 * k1 + 1]
        # [B,H,16blk,32u,4j] -> [B,H,blk,4j,32u] -> [TROWS, 128]
        tk = ref_q[:, :, :, sel].reshape(B, H, W // UNITS, UNITS, 4)
        tk = np.ascontiguousarray(tk.transpose(0, 1, 2, 4, 3))
        tables[f"t{k}"] = tk.reshape(TROWS, 128)

    # weights along m-sorted cols: wv[p, (s'*4+nh)*4 + c] = [w0,w0,w1,w1]
    w0s, w1s = wts[0][sperm], wts[1][sperm]                # [S] in s' order
    wvrow = np.stack([w0s, w0s, w1s, w1s], -1)             # [S,4]
    wvrow = np.repeat(wvrow[:, None, :], NH, axis=1).reshape(-1)   # [S*NH*4]
    wvfull = np.ascontiguousarray(np.broadcast_to(wvrow, (128, COLS * 4))
                                  .astype(np.float16))

    iota32 = np.ascontiguousarray(
        np.broadcast_to(np.arange(UNITS, dtype=np.float16), (128, UNITS)))

    in_maps = []
    HCOLS = COLS // 2
    for core in range(NCORES):
        n0 = core * ND
        dsl = det[n0:n0 + ND]                              # [ND,S,2]
        # [p, s'*4+nh] = det[n0 + nh*128 + p, sperm[s'], j]
        dn = dsl.reshape(NH, 128, S, 2)[:, :, sperm, :]    # [nh,p,s',2]
        dxh = np.ascontiguousarray(dn[..., 0].transpose(1, 2, 0).reshape(128, COLS))
        dyh = np.ascontiguousarray(dn[..., 1].transpose(1, 2, 0).reshape(128, COLS))
        bco = bb[n0:n0 + ND].reshape(NH, 128)              # [nh,p]
        bBh = np.ascontiguousarray(
            (bco.T * ROWS_PER_B).astype(np.float32))       # [p,nh]

        # host-computed iteration-0 prep (round RNE + clip + index math +
        # the 16-partition wrapped/replicated gather-row layout)
        rx0 = np.clip(np.rint(dxh.astype(np.float32)), 0, W - 1).astype(np.float32)
        ry0 = np.clip(np.rint(dyh.astype(np.float32)), 0, H - 1).astype(np.float32)
        px0 = ry0 * np.float32(W) + rx0
        fl0 = np.floor(px0 / 32).astype(np.float32)
        row0 = fl0 + bBh[:, np.tile(np.arange(NH), S)]     # [p, col]
        uu0 = (px0 - 32 * fl0).astype(np.float16)
        idx0 = row0.astype(np.int16)
        cmap = {}
        for hf in range(2):
            sl = slice(hf * HCOLS, (hf + 1) * HCOLS)
            idxh = idx0[:, sl]                             # [128, HCOLS]
            wrp = np.empty((16, HCOLS * 8), np.int16)
            for sh in range(8):
                wrp[:, sh::8] = idxh[sh * 16:(sh + 1) * 16, :]
            cmap[f"rx0_{hf}"] = np.ascontiguousarray(rx0[:, sl])
            cmap[f"ry0_{hf}"] = np.ascontiguousarray(ry0[:, sl])
            cmap[f"uu0_{hf}"] = np.ascontiguousarray(uu0[:, sl])
            cmap[f"wrp0_{hf}"] = np.ascontiguousarray(np.tile(wrp, (8, 1)))
        in_maps.append({**tables, **cmap,
                        "bB": bBh, "wv": wvfull, "iota32": iota32})
    return in_maps, tuple(bounds), sperm


def assemble_output(results, sperm):
    inv = np.empty_like(sperm)
    inv[sperm] = np.arange(S)
    out = np.empty((N, S, 2), np.float32)
    for core in range(NCORES):
        n0 = core * ND
        ox = results[core]["outx"].reshape(128, S, NH)     # [p,s',nh]
        oy = results[core]["outy"].reshape(128, S, NH)
        # out[n0 + nh*128 + p, s, j]
        out[n0:n0 + ND, :, 0] = ox.transpose(2, 0, 1).reshape(ND, S)[:, inv]
        out[n0:n0 + ND, :, 1] = oy.transpose(2, 0, 1).reshape(ND, S)[:, inv]
    return out


def get_nc(bounds):
    if bounds not in _cache:
        _cache[bounds] = _build_nc(bounds)
    return _cache[bounds]


def kernel(det_indices, refinement, sampling, b,
           num_loops=4, num_buckets=6, height=512, width=512, **_kw):
    assert int(num_loops) == NL and int(num_buckets) == NB
    assert int(height) == H and int(width) == W
    in_maps, bounds, sperm = prep_inputs(det_indices, refinement, sampling, b)
    nc = get_nc(bounds)
    res = run_bass_kernel_spmd(nc, in_maps, core_ids=list(range(NCORES)))
    return assemble_output(res.results, sperm)
